# revision 26
# baseline (speedup 1.0000x reference)
"""GQA kernel for 8x TRN2 NeuronCores (Bass/Tile), DP2 x TP4 sharding.

Layout strategy (per core; batch b = core//4, shard t = core%4):
  - x fed transposed (feature-major) xT [D, S]; projections emit token-major
    q/k/v and feature-major gate^T via PE matmuls.
  - rmsnorm+rope token-major (free-dim reductions), then PE-transpose q,k to
    feature-major for attention. rstd is applied AFTER rope (it commutes: a
    per-(token,head) scalar), so psum frees early and Rsqrt batches per chunk.
  - gate^T stays in SBUF (graw, bf16); sigmoid applied in-place, batched per
    512-token chunk, so the ACT table only cycles Rsqrt/Sigmoid/Exp once per
    chunk instead of per tile.
  - scores^T [k,128 x q,512] blocks = kT.T @ qT (K=64); exp on ScalarE; causal
    diagonal blocks trimmed to q >= kt*128 (matmul, exp and mask-mul all start
    at column lo), masked via one shared [128,128] lower-triangle tile.
  - ctx^T accumulated feature-major with v_ext=[v|ones] so softmax sums come
    free; for group 1 the matmul writes psum partitions 63..127 (ones col
    first) so the normalize/gate multiplies are partition-aligned with the
    upper half of ctxg -- no cross-partition DMA.
  - out projection token-major with ctxg stationary; y emitted bf16; partial
    [S, D] outputs summed across the 4 TP shards on host.
Local head order is interleaved (0,4,1,5,2,6,3,7) so transposed q tiles put a
g0 head on partitions 0-63 and a g1 head on 64-127, matching kT/gate/Wo
layouts without any cross-partition moves.
"""
import sys

if "/opt/trn_rl_repo" not in sys.path:
    sys.path.insert(0, "/opt/trn_rl_repo")

import numpy as np

import concourse.bass as bass
import concourse.mybir as mybir
import concourse.tile as tile
from concourse import bacc

B, S, D = 2, 2048, 2048
H, G, HD = 32, 8, 64
EPS = 1e-6
NCORES = 8
NT = S // 128          # 16 s-tiles
NQC = S // 512         # 4 q-chunks
F32 = mybir.dt.float32
BF16 = mybir.dt.bfloat16

_PERM = [0, 4, 1, 5, 2, 6, 3, 7]  # local head order (token-major col blocks)


def _bc(ap, n, where="last"):
    """stride-0 broadcast dim appended (or inserted after partition dim)."""
    if where == "last":
        return bass.AP(tensor=ap.tensor, offset=ap.offset, ap=[*ap.ap, [0, n]])
    return bass.AP(tensor=ap.tensor, offset=ap.offset,
                   ap=[ap.ap[0], [0, n], *ap.ap[1:]])


def classify_mask(mask):
    """Per (qc, kt) block class for scores^T blocks.

    Returns (classes, tiles) where classes[qc][kt] is one of
      'skip'            -- fully masked block
      ('clean',)        -- no masking
      ('tri', lo)       -- causal diagonal block: cols < lo fully masked,
                           cols [lo, lo+128) lower-triangle, rest clean
      ('mask', idx)     -- general: multiply full width by tiles[idx]
    """
    classes = []
    tiles = []
    keyidx = {}
    q_loc = np.arange(512)[:, None]
    k_loc = np.arange(128)[None, :]
    for qc in range(NQC):
        row = []
        for kt in range(NT):
            sub = mask[qc * 512:(qc + 1) * 512, kt * 128:(kt + 1) * 128]
            if sub.all():
                row.append("skip")
            elif not sub.any():
                row.append(("clean",))
            else:
                lo = kt * 128 - qc * 512
                if 0 <= lo <= 384:
                    expect = (q_loc - lo) < k_loc  # True = masked
                    if np.array_equal(sub, expect):
                        row.append(("tri", lo))
                        continue
                t = (~sub.T).astype(np.float32)  # [128k, 512q] 1=keep
                key = t.tobytes()
                if key not in keyidx:
                    keyidx[key] = len(tiles)
                    tiles.append(t)
                row.append(("mask", keyidx[key]))
        classes.append(row)
    return classes, tiles


def build_program(classes, n_masks):
    nc = bacc.Bacc("TRN2", target_bir_lowering=False, debug=False)

    def mm(out, lhsT, rhs, start, stop):
        nc.tensor.matmul(out, lhsT=lhsT, rhs=rhs, start=start, stop=stop)

    xT = nc.dram_tensor("xT", [D, S], BF16, kind="ExternalInput")
    wq = nc.dram_tensor("wq", [D, 512], BF16, kind="ExternalInput")
    wkv = nc.dram_tensor("wkv", [D, 256], BF16, kind="ExternalInput")
    wg = nc.dram_tensor("wg", [D, 512], BF16, kind="ExternalInput")
    wo = nc.dram_tensor("wo", [512, D], BF16, kind="ExternalInput")
    cosd = nc.dram_tensor("cosd", [S, HD], BF16, kind="ExternalInput")
    sind = nc.dram_tensor("sind", [S, HD], BF16, kind="ExternalInput")
    qsc = nc.dram_tensor("qsc", [128, 512], F32, kind="ExternalInput")
    ksc = nc.dram_tensor("ksc", [128, 128], F32, kind="ExternalInput")
    trit = nc.dram_tensor("trit", [128, 128], BF16, kind="ExternalInput")
    ident_in = nc.dram_tensor("ident", [128, 128], BF16, kind="ExternalInput")
    if n_masks:
        maskt = nc.dram_tensor("maskt", [n_masks, 128, 512], BF16,
                               kind="ExternalInput")
    y = nc.dram_tensor("y", [S, D], BF16, kind="ExternalOutput")

    AF = mybir.ActivationFunctionType
    from contextlib import ExitStack
    with tile.TileContext(nc) as tc, ExitStack() as es:
        singles = es.enter_context(tc.tile_pool(name="singles", bufs=1))
        xpool = es.enter_context(tc.tile_pool(name="xpool", bufs=2))
        pwork = es.enter_context(tc.tile_pool(name="pwork", bufs=2))
        psum = es.enter_context(tc.tile_pool(name="psum", bufs=1, space="PSUM"))
        awork = es.enter_context(tc.tile_pool(name="awork", bufs=3, space="SBUF"))

        # ---- resident constants / weights; order = startup DMA priority ----
        wq_sb = singles.tile([128, NT, 512], BF16)
        wq_r = wq.ap().rearrange("(a p) n -> p a n", p=128)
        nc.sync.dma_start(out=wq_sb[:, 0:8, :], in_=wq_r[:, 0:8, :])

        xch = {}

        def load_xchunk(c):
            t = xpool.tile([128, NT, 256], BF16, tag="xch", name=f"xch_{c}")
            nc.sync.dma_start(
                out=t, in_=xT.ap()[:, c * 256:(c + 1) * 256]
                .rearrange("(a p) m -> p a m", p=128))
            xch[c] = t

        load_xchunk(0)
        nc.sync.dma_start(out=wq_sb[:, 8:16, :], in_=wq_r[:, 8:16, :])
        qsc_sb = singles.tile([128, 512], F32)
        nc.sync.dma_start(out=qsc_sb, in_=qsc.ap())
        ksc_sb = singles.tile([128, 128], F32)
        nc.sync.dma_start(out=ksc_sb, in_=ksc.ap())
        wkv_sb = singles.tile([128, NT, 256], BF16)
        nc.sync.dma_start(out=wkv_sb, in_=wkv.ap().rearrange("(a p) n -> p a n", p=128))
        load_xchunk(1)
        cos_sb = singles.tile([128, NT, HD], BF16)
        nc.sync.dma_start(out=cos_sb, in_=cosd.ap().rearrange("(a p) n -> p a n", p=128))
        sin_sb = singles.tile([128, NT, HD], BF16)
        nc.sync.dma_start(out=sin_sb, in_=sind.ap().rearrange("(a p) n -> p a n", p=128))
        ident_sb = singles.tile([128, 128], BF16)
        nc.sync.dma_start(out=ident_sb, in_=ident_in.ap())
        tri_sb = singles.tile([128, 128], BF16)
        nc.sync.dma_start(out=tri_sb, in_=trit.ap())
        wg_sb = singles.tile([128, NT, 512], BF16)
        nc.sync.dma_start(out=wg_sb, in_=wg.ap().rearrange("(a p) n -> p a n", p=128))
        if n_masks:
            mask_sb = singles.tile([128, n_masks, 512], BF16)
            nc.sync.dma_start(out=mask_sb,
                              in_=maskt.ap().rearrange("a p n -> p a n"))
        # wo load deferred (emitted before the first out-projection) so it
        # does not block the x-chunk streaming DMAs at startup
        wo_sb = singles.tile([128, 4, D], BF16)
        wo_loaded = []

        def load_wo():
            if not wo_loaded:
                nc.sync.dma_start(
                    out=wo_sb, in_=wo.ap().rearrange("(a p) n -> p a n", p=128))
                wo_loaded.append(True)

        qT = singles.tile([128, 4, S], BF16)       # head nt @0-63, 4+nt @64-127
        kT = singles.tile([128, S], BF16)          # group0 @0-63, group1 @64-127
        vext = singles.tile([128, 2, NT, 65], BF16)  # [v(64) | ones]
        nc.vector.memset(vext[:, :, :, 64], 1.0)
        eps_sb = singles.tile([128, 1], F32)
        nc.vector.memset(eps_sb, float(EPS))
        ones_sb = singles.tile([128, 64], BF16)
        nc.vector.memset(ones_sb, 1.0)
        graw = singles.tile([128, 4, S], BF16)     # gate^T; sigmoid in-place

        def emit_g_chains(i, xt):
            for nt in range(4):
                g_ps = psum.tile([128, 128], F32, tag="ps_to", bufs=2,
                                 name=f"gps_{i}_{nt}")
                for dt_ in range(NT):
                    mm(g_ps, wg_sb[:, dt_, nt * 128:(nt + 1) * 128],
                       rhs=xt[:, dt_, :],
                       start=(dt_ == 0), stop=(dt_ == NT - 1))
                nc.scalar.copy(graw[:, nt, i * 128:(i + 1) * 128], g_ps)

        def phase_p_tile(i, defer_g=False):
            """projections + rmsnorm + rope (rstd deferred) for s-tile i."""
            c, half = i // 2, i % 2
            if half == 0 and c + 1 < S // 256 and (c + 1) not in xch:
                load_xchunk(c + 1)
            xt = xch[c][:, :, half * 128:(half + 1) * 128]

            q_ps = psum.tile([128, 512], F32, tag="ps_qp", bufs=1, name=f"qps_{i}")
            for dt_ in range(NT):
                mm(q_ps, xt[:, dt_, :], rhs=wq_sb[:, dt_, :],
                   start=(dt_ == 0), stop=(dt_ == NT - 1))
            kv_ps = psum.tile([128, 256], F32, tag="ps_to", bufs=2, name=f"kvps_{i}")
            for dt_ in range(NT):
                mm(kv_ps, xt[:, dt_, :], rhs=wkv_sb[:, dt_, :],
                   start=(dt_ == 0), stop=(dt_ == NT - 1))
            if not defer_g:
                emit_g_chains(i, xt)

            # ---- q: square+reduce (rstd later), scale, rope ----
            q3 = q_ps.rearrange("p (h e) -> p h e", e=64)
            sq = pwork.tile([128, 8, 64], BF16, tag="sq")
            nc.scalar.square(sq, q3)
            nc.vector.reduce_sum(ssq[:, i % 4, :], sq, axis=mybir.AxisListType.X)
            qn = pwork.tile([128, 8, 64], BF16, tag="qn")
            nc.vector.tensor_mul(qn.rearrange("p h e -> p (h e)"), q_ps, qsc_sb)
            rot = pwork.tile([128, 8, 64], BF16, tag="rot")
            nc.vector.tensor_scalar_mul(rot[:, :, 0:32], qn[:, :, 32:64], -1.0)
            nc.vector.tensor_copy(rot[:, :, 32:64], qn[:, :, 0:32])
            qf = qfc[i % 4]
            for h in range(8):
                nc.vector.tensor_mul(qf[:, h, :], qn[:, h, :], cos_sb[:, i, :])
                nc.vector.tensor_mul(rot[:, h, :], rot[:, h, :], sin_sb[:, i, :])
            nc.vector.tensor_add(qf.rearrange("p h e -> p (h e)"),
                                 qf.rearrange("p h e -> p (h e)"),
                                 rot.rearrange("p h e -> p (h e)"))

            # ---- k ----
            k3 = kv_ps[:, 0:128].rearrange("p (h e) -> p h e", e=64)
            ksq = pwork.tile([128, 2, 64], BF16, tag="ksq")
            nc.scalar.square(ksq, k3)
            nc.vector.reduce_sum(ssk[:, i % 4, :], ksq, axis=mybir.AxisListType.X)
            kn = pwork.tile([128, 2, 64], BF16, tag="kn")
            nc.vector.tensor_mul(kn.rearrange("p h e -> p (h e)"),
                                 kv_ps[:, 0:128], ksc_sb)
            krot = pwork.tile([128, 2, 64], BF16, tag="krot")
            nc.vector.tensor_scalar_mul(krot[:, :, 0:32], kn[:, :, 32:64], -1.0)
            nc.vector.tensor_copy(krot[:, :, 32:64], kn[:, :, 0:32])
            kf = kfc[i % 4]
            for h in range(2):
                nc.vector.tensor_mul(kf[:, h, :], kn[:, h, :], cos_sb[:, i, :])
                nc.vector.tensor_mul(krot[:, h, :], krot[:, h, :], sin_sb[:, i, :])
            nc.vector.tensor_add(kf.rearrange("p h e -> p (h e)"),
                                 kf.rearrange("p h e -> p (h e)"),
                                 krot.rearrange("p h e -> p (h e)"))

            # v into v_ext (cast to bf16)
            nc.scalar.copy(vext[:, 0, i, 0:64], kv_ps[:, 128:192])
            nc.scalar.copy(vext[:, 1, i, 0:64], kv_ps[:, 192:256])
            return xt

        def phase_p_chunk_tail(qc):
            """sigmoid + batched rsqrt + rstd application + transposes."""
            # gate sigmoid first: depends only on graw, and the g1-low DMA
            # (engines cannot cross partitions) comes off the critical path
            gsl = graw[:, :, qc * 512:(qc + 1) * 512]
            nc.scalar.activation(gsl, gsl, AF.Sigmoid)
            gl = awork.tile([64, 4, 512], BF16, tag="glow", bufs=2,
                            name=f"glow_{qc}")
            nc.sync.dma_start(out=gl,
                              in_=graw[64:128, :, qc * 512:(qc + 1) * 512])
            nc.scalar.activation(ssq, ssq, AF.Sqrt, bias=eps_sb, scale=1.0 / 64)
            nc.scalar.activation(ssk, ssk, AF.Sqrt, bias=eps_sb, scale=1.0 / 64)
            nc.vector.reciprocal(ssq, ssq)
            nc.vector.reciprocal(ssk, ssk)
            for u in range(4):
                i = qc * 4 + u
                qf, kf = qfc[u], kfc[u]
                for h in range(8):
                    nc.vector.tensor_scalar_mul(qf[:, h, :], qf[:, h, :],
                                                ssq[:, u, h:h + 1])
                for h in range(2):
                    nc.vector.tensor_scalar_mul(kf[:, h, :], kf[:, h, :],
                                                ssk[:, u, h:h + 1])
                qf2 = qf.rearrange("p h e -> p (h e)")
                for nt in range(4):
                    tp = psum.tile([128, 128], BF16, tag="ps_s", bufs=3,
                                   name=f"tp_{i}_{nt}")
                    nc.tensor.transpose(tp, qf2[:, nt * 128:(nt + 1) * 128],
                                        ident_sb)
                    nc.scalar.copy(qT[:, nt, i * 128:(i + 1) * 128], tp)
                kf2 = kf.rearrange("p h e -> p (h e)")
                tpk = psum.tile([128, 128], BF16, tag="ps_s", bufs=3,
                                name=f"tpk_{i}")
                nc.tensor.transpose(tpk, kf2, ident_sb)
                nc.scalar.copy(kT[:, i * 128:(i + 1) * 128], tpk)
            return gl

        def a_head(qc, h, ctxg, gl):
            """attention for one head of q-chunk qc."""
            g, nt = h // 4, h % 4
            base = 64 * g
            kts = [kt for kt in range(NT) if classes[qc][kt] != "skip"]
            ctx_ps = psum.tile([128, 512], F32, tag="ps_ctx", bufs=2,
                               name=f"ctx_{qc}_{h}")

            def emit_score(j):
                kt = kts[j]
                cls = classes[qc][kt]
                lo = cls[1] if cls[0] == "tri" else 0
                s_ps = psum.tile([128, 512], F32, tag="ps_s", bufs=3,
                                 name=f"sps_{qc}_{h}_{kt}")
                mm(s_ps[:, lo:512],
                   kT[base:base + 64, kt * 128:(kt + 1) * 128],
                   rhs=qT[base:base + 64, nt, qc * 512 + lo:(qc + 1) * 512],
                   start=True, stop=True)
                eT = awork.tile([128, 512], BF16, tag="eT", bufs=4)
                nc.scalar.activation(eT[:, lo:512], s_ps[:, lo:512], AF.Exp)
                if cls[0] == "tri":
                    nc.vector.tensor_mul(eT[:, lo:lo + 128],
                                         eT[:, lo:lo + 128], tri_sb)
                elif cls[0] == "mask":
                    nc.vector.tensor_mul(eT, eT, mask_sb[:, cls[1], :])
                return eT, lo

            def emit_ctx(j, eT, lo):
                mm(ctx_ps[0:65, lo:512], vext[:, g, kts[j], :],
                   rhs=eT[:, lo:512], start=(j == 0), stop=(j == len(kts) - 1))

            # scores emitted one step ahead of the ctx accumulation so the
            # PE stream never head-of-line blocks on an exp in flight
            pend = None
            for j in range(len(kts)):
                cur = emit_score(j)
                if pend is not None:
                    emit_ctx(j - 1, *pend)
                pend = cur
            emit_ctx(len(kts) - 1, *pend)

            def normalize():
                # normalize + gate (partitions 0-63; denom at row 64)
                rstage = awork.tile([128, 512], BF16, tag="rstage", bufs=2)
                with nc.allow_low_precision(reason="bf16 softmax denom"):
                    nc.vector.reciprocal(rstage[64:65, :], ctx_ps[64:65, :])
                rb_ps = psum.tile([128, 512], F32, tag="ps_s", bufs=3,
                                  name=f"rbps_{qc}_{h}")
                mm(rb_ps[0:64, :], ones_sb[64:65, :],
                   rhs=rstage[64:65, :], start=True, stop=True)
                rbb = awork.tile([64, 512], BF16, tag="rbb", bufs=2)
                nc.vector.tensor_copy(rbb, rb_ps[0:64, :])
                ctxb = awork.tile([64, 512], BF16, tag="ctxb", bufs=2)
                nc.vector.tensor_copy(ctxb, ctx_ps[0:64, :])
                m1 = awork.tile([128, 512], BF16, tag="m1", bufs=2)
                gsl = (gl[:, nt, :] if g == 1
                       else graw[0:64, nt, qc * 512:(qc + 1) * 512])
                nc.vector.tensor_mul(m1[0:64, :], rbb, gsl)
                if g == 0:
                    nc.vector.tensor_mul(ctxg[nt][0:64, :], ctxb, m1[0:64, :])
                else:
                    tmp2 = awork.tile([64, 512], BF16, tag="tmp2", bufs=2)
                    nc.vector.tensor_mul(tmp2, ctxb, m1[0:64, :])
                    nc.sync.dma_start(out=ctxg[nt][64:128, :], in_=tmp2)
            return normalize

        def a_outproj_ssub(qc, ctxg, ssub):
            """output projection for one 128-row slab of q-chunk qc."""
            srow = qc * 512 + ssub * 128
            ostage = awork.tile([128, D], BF16, tag="ostage", bufs=3)
            for dc in range(4):
                o_ps = psum.tile([128, 512], F32, tag="ps_to", bufs=2,
                                 name=f"ops_{qc}_{ssub}_{dc}")
                for nt in range(4):
                    mm(o_ps, ctxg[nt][:, ssub * 128:(ssub + 1) * 128],
                       rhs=wo_sb[:, nt, dc * 512:(dc + 1) * 512],
                       start=(nt == 0), stop=(nt == 3))
                # last chunk runs after all exps: ACT is idle, share the
                # psum drains between both engines there
                if qc == NQC - 1 and dc % 2 == 0:
                    nc.scalar.copy(ostage[:, dc * 512:(dc + 1) * 512], o_ps)
                else:
                    nc.vector.tensor_copy(ostage[:, dc * 512:(dc + 1) * 512],
                                          o_ps)
            nc.sync.dma_start(out=y.ap()[srow:srow + 128, :], in_=ostage)

        # ======== main schedule: P chunk -> (tail) -> A chunk, interleaved ==
        ssq = singles.tile([128, 4, 8], F32)
        ssk = singles.tile([128, 4, 2], F32)
        qfc = [singles.tile([128, 8, 64], BF16, name=f"qfc{u}") for u in range(4)]
        kfc = [singles.tile([128, 2, 64], BF16, name=f"kfc{u}") for u in range(4)]

        # A(qc) heads interleaved with P tiles of chunk qc+1 and the
        # out-projection slabs of chunk qc-1, so the PE always has dense
        # projection matmuls to chew on while ACT runs the exps. g1 heads
        # first: their ctxg partition-shift DMAs overlap the later heads.
        # Chunk 0's gate chains are deferred past the q/kv chains so the PE
        # is not stalled on the wg weight DMA at startup.
        xts0 = [phase_p_tile(u, defer_g=True) for u in range(4)]
        for u in range(4):
            emit_g_chains(u, xts0[u])
        gl = phase_p_chunk_tail(0)
        HORDER = (4, 5, 6, 7, 0, 1, 2, 3)
        prev = None  # (qc, ctxg) awaiting out-projection
        for qc in range(NQC):
            load_wo()
            ctxg = [awork.tile([128, 512], BF16, tag=f"ctxg{nt}",
                               name=f"ctxg{nt}_{qc}", bufs=2)
                    for nt in range(4)]
            pend_norm = None
            for u, h in enumerate(HORDER):
                norm = a_head(qc, h, ctxg, gl)
                if pend_norm is not None:
                    pend_norm()
                pend_norm = norm
                if qc + 1 < NQC and u < 4:
                    phase_p_tile((qc + 1) * 4 + u)
                if prev is not None and u % 2 == 1:
                    a_outproj_ssub(prev[0], prev[1], u // 2)
            pend_norm()
            # tail of the NEXT chunk ahead of the trailing out-projections:
            # they are PE-dense with no ACT work, hiding the whole
            # sigmoid/sqrt/transpose chain (and its table switches)
            if qc + 1 < NQC:
                gl = phase_p_chunk_tail(qc + 1)
            prev = (qc, ctxg)
        for ssub in range(4):
            a_outproj_ssub(prev[0], prev[1], ssub)

    nc.compile()
    return nc


# ======================== host-side runner =================================
_CACHE = {}


class _Runner:
    """Jitted sharded executable for a prebuilt Bass module, reusable."""

    def __init__(self, nc, n_cores):
        import jax
        import numpy as _np
        from jax.sharding import Mesh, PartitionSpec
        from jax.experimental.shard_map import shard_map
        from concourse.bass2jax import (_bass_exec_p, partition_id_tensor,
                                        install_neuronx_cc_hook)
        install_neuronx_cc_hook()
        self.jax = jax
        self.nc = nc
        self.n_cores = n_cores
        partition_name = (nc.partition_id_tensor.name
                          if nc.partition_id_tensor else None)
        in_names, out_names, out_avals = [], [], []
        for alloc in nc.m.functions[0].allocations:
            if not isinstance(alloc, mybir.MemoryLocationSet):
                continue
            name = alloc.memorylocations[0].name
            if alloc.kind == "ExternalInput":
                if name != partition_name:
                    in_names.append(name)
            elif alloc.kind == "ExternalOutput":
                out_names.append(name)
                out_avals.append(jax.core.ShapedArray(
                    tuple(alloc.tensor_shape), mybir.dt.np(alloc.dtype)))
        self.in_names, self.out_names, self.out_avals = in_names, out_names, out_avals
        n_params = len(in_names)
        all_in = list(in_names) + list(out_names)
        if partition_name is not None:
            all_in.append(partition_name)
        self._dbg_name = nc.dbg_addr.name if nc.dbg_addr is not None else None

        def _body(*args):
            operands = list(args)
            if partition_name is not None:
                operands.append(partition_id_tensor())
            outs = _bass_exec_p.bind(
                *operands, out_avals=tuple(out_avals), in_names=tuple(all_in),
                out_names=tuple(out_names), lowering_input_output_aliases=(),
                sim_require_finite=True, sim_require_nnan=True, nc=nc)
            return tuple(outs)

        devices = jax.devices()[:n_cores]
        self.mesh = Mesh(_np.asarray(devices), ("core",))
        in_specs = (PartitionSpec("core"),) * (n_params + len(out_names))
        out_specs = (PartitionSpec("core"),) * len(out_names)
        self.fn = jax.jit(shard_map(_body, mesh=self.mesh, in_specs=in_specs,
                                    out_specs=out_specs, check_rep=False))
        self.dev_in = None

    def prepare(self, in_maps):
        import numpy as _np
        from jax.sharding import NamedSharding, PartitionSpec
        if self._dbg_name is not None:
            in_maps = [{**m, self._dbg_name: _np.zeros((1, 2), _np.uint32)}
                       for m in in_maps]
        concat = [_np.concatenate([_np.asarray(in_maps[c][n])
                                   for c in range(self.n_cores)], axis=0)
                  for n in self.in_names]
        # zero output buffers: device-resident, NOT donated, reused each run.
        # Valid because the kernel writes every element of its outputs.
        concat += [_np.zeros((self.n_cores * av.shape[0], *av.shape[1:]),
                             av.dtype) for av in self.out_avals]
        shard = NamedSharding(self.mesh, PartitionSpec("core"))
        self.dev_in = [self.jax.device_put(a, shard) for a in concat]
        return self

    def run(self):
        return self.jax.block_until_ready(self.fn(*self.dev_in))

    def results(self, outs):
        import numpy as _np
        res = []
        for c in range(self.n_cores):
            d = {}
            for i, name in enumerate(self.out_names):
                full = _np.asarray(outs[i])
                d[name] = full.reshape(self.n_cores, *self.out_avals[i].shape)[c]
            res.append(d)
        return res


def make_runner(nc, n_cores):
    return _Runner(nc, n_cores)


def _prep_core_inputs(inputs, b, t, shared):
    x = inputs["x"]
    import ml_dtypes
    bf = ml_dtypes.bfloat16

    if ("xT", b) not in shared:
        shared[("xT", b)] = np.ascontiguousarray(np.asarray(x[b]).T).astype(bf)
    if ("w", t) not in shared:
        Wq, Wk, Wv, Wg, Wo = (np.asarray(inputs[k])
                              for k in ("Wq", "Wk", "Wv", "Wg", "Wo"))
        heads = [8 * t + p for p in _PERM]
        qcols = np.concatenate([np.arange(h * 64, (h + 1) * 64) for h in heads])
        groups = [2 * t, 2 * t + 1]
        kcols = np.concatenate([np.arange(g * 64, (g + 1) * 64) for g in groups])
        shared[("w", t)] = {
            "wq": np.ascontiguousarray(Wq[:, qcols]).astype(bf),
            "wkv": np.ascontiguousarray(
                np.concatenate([Wk[:, kcols], Wv[:, kcols]], axis=1)).astype(bf),
            "wg": np.ascontiguousarray(Wg[:, qcols]).astype(bf),
            "wo": np.ascontiguousarray(Wo[qcols, :]).astype(bf),
        }
    if "const" not in shared:
        q_scale, k_scale = np.asarray(inputs["q_scale"]), np.asarray(inputs["k_scale"])
        cos, sin = np.asarray(inputs["cos"]), np.asarray(inputs["sin"])
        scaling = float(HD) ** -0.5
        tri = (np.arange(128)[:, None] <= np.arange(128)[None, :])
        shared["const"] = {
            "cosd": cos.astype(bf), "sind": sin.astype(bf),
            "qsc": np.broadcast_to(
                np.tile((1.0 + q_scale) * scaling, 8)[None, :],
                (128, 512)).astype(np.float32).copy(),
            "ksc": np.broadcast_to(
                np.tile(1.0 + k_scale, 2)[None, :],
                (128, 128)).astype(np.float32).copy(),
            "trit": tri.astype(bf),
            "ident": np.eye(128, dtype=np.float32).astype(bf),
        }
    return {"xT": shared[("xT", b)], **shared[("w", t)], **shared["const"]}


def kernel(**inputs):
    mask = np.asarray(inputs["mask"])
    classes, tiles = classify_mask(mask)
    key = mask.tobytes()
    if key not in _CACHE:
        nc = build_program(classes, len(tiles))
        _CACHE[key] = (nc, make_runner(nc, NCORES))
    nc, runner = _CACHE[key]

    import ml_dtypes
    mask_arr = (np.stack(tiles).astype(ml_dtypes.bfloat16) if tiles else None)
    shared = {}
    in_maps = []
    for c in range(NCORES):
        m = _prep_core_inputs(inputs, c // 4, c % 4, shared)
        if mask_arr is not None:
            m["maskt"] = mask_arr
        in_maps.append(m)

    runner.prepare(in_maps)
    outs = runner.run()
    res = runner.results(outs)
    out = np.zeros((B, S, D), np.float32)
    for c in range(NCORES):
        out[c // 4] += res[c]["y"].astype(np.float32)
    return out.astype(np.asarray(inputs["x"]).dtype)


# revision 29
# speedup vs baseline: 1.0410x; 1.0410x over previous
"""GQA kernel for 8x TRN2 NeuronCores (Bass/Tile), DP2 x TP4 sharding.

Layout strategy (per core; batch b = core//4, shard t = core%4):
  - x fed transposed (feature-major) xT [D, S]; projections emit token-major
    q/k/v and feature-major gate^T via PE matmuls.
  - rmsnorm+rope token-major (free-dim reductions), then PE-transpose q,k to
    feature-major for attention. rstd is applied AFTER rope (it commutes: a
    per-(token,head) scalar), so psum frees early and Rsqrt batches per chunk.
  - gate^T stays in SBUF (graw, bf16); sigmoid applied in-place, batched per
    512-token chunk, so the ACT table only cycles Rsqrt/Sigmoid/Exp once per
    chunk instead of per tile.
  - scores^T [k,128 x q,512] blocks = kT.T @ qT (K=64); exp on ScalarE; causal
    diagonal blocks trimmed to q >= kt*128 (matmul, exp and mask-mul all start
    at column lo), masked via one shared [128,128] lower-triangle tile.
  - ctx^T accumulated feature-major with v_ext=[v|ones] so softmax sums come
    free; for group 1 the matmul writes psum partitions 63..127 (ones col
    first) so the normalize/gate multiplies are partition-aligned with the
    upper half of ctxg -- no cross-partition DMA.
  - out projection token-major with ctxg stationary; y emitted bf16; partial
    [S, D] outputs summed across the 4 TP shards on host.
Local head order is interleaved (0,4,1,5,2,6,3,7) so transposed q tiles put a
g0 head on partitions 0-63 and a g1 head on 64-127, matching kT/gate/Wo
layouts without any cross-partition moves.
"""
import sys

if "/opt/trn_rl_repo" not in sys.path:
    sys.path.insert(0, "/opt/trn_rl_repo")

import numpy as np

import concourse.bass as bass
import concourse.mybir as mybir
import concourse.tile as tile
from concourse import bacc

B, S, D = 2, 2048, 2048
H, G, HD = 32, 8, 64
EPS = 1e-6
NCORES = 8
NT = S // 128          # 16 s-tiles
NQC = S // 512         # 4 q-chunks
F32 = mybir.dt.float32
BF16 = mybir.dt.bfloat16

_PERM = [0, 4, 1, 5, 2, 6, 3, 7]  # local head order (token-major col blocks)


def _bc(ap, n, where="last"):
    """stride-0 broadcast dim appended (or inserted after partition dim)."""
    if where == "last":
        return bass.AP(tensor=ap.tensor, offset=ap.offset, ap=[*ap.ap, [0, n]])
    return bass.AP(tensor=ap.tensor, offset=ap.offset,
                   ap=[ap.ap[0], [0, n], *ap.ap[1:]])


def classify_mask(mask):
    """Per (qc, kt) block class for scores^T blocks.

    Returns (classes, tiles) where classes[qc][kt] is one of
      'skip'            -- fully masked block
      ('clean',)        -- no masking
      ('tri', lo)       -- causal diagonal block: cols < lo fully masked,
                           cols [lo, lo+128) lower-triangle, rest clean
      ('mask', idx)     -- general: multiply full width by tiles[idx]
    """
    classes = []
    tiles = []
    keyidx = {}
    q_loc = np.arange(512)[:, None]
    k_loc = np.arange(128)[None, :]
    for qc in range(NQC):
        row = []
        for kt in range(NT):
            sub = mask[qc * 512:(qc + 1) * 512, kt * 128:(kt + 1) * 128]
            if sub.all():
                row.append("skip")
            elif not sub.any():
                row.append(("clean",))
            else:
                lo = kt * 128 - qc * 512
                if 0 <= lo <= 384:
                    expect = (q_loc - lo) < k_loc  # True = masked
                    if np.array_equal(sub, expect):
                        row.append(("tri", lo))
                        continue
                t = (~sub.T).astype(np.float32)  # [128k, 512q] 1=keep
                key = t.tobytes()
                if key not in keyidx:
                    keyidx[key] = len(tiles)
                    tiles.append(t)
                row.append(("mask", keyidx[key]))
        classes.append(row)
    return classes, tiles


def build_program(classes, n_masks):
    nc = bacc.Bacc("TRN2", target_bir_lowering=False, debug=False)

    def mm(out, lhsT, rhs, start, stop):
        nc.tensor.matmul(out, lhsT=lhsT, rhs=rhs, start=start, stop=stop)

    xT = nc.dram_tensor("xT", [D, S], BF16, kind="ExternalInput")
    wq = nc.dram_tensor("wq", [D, 512], BF16, kind="ExternalInput")
    wkv = nc.dram_tensor("wkv", [D, 256], BF16, kind="ExternalInput")
    wg = nc.dram_tensor("wg", [D, 512], BF16, kind="ExternalInput")
    wo = nc.dram_tensor("wo", [512, D], BF16, kind="ExternalInput")
    cosd = nc.dram_tensor("cosd", [S, HD], BF16, kind="ExternalInput")
    sind = nc.dram_tensor("sind", [S, HD], BF16, kind="ExternalInput")
    qsc = nc.dram_tensor("qsc", [128, 512], F32, kind="ExternalInput")
    ksc = nc.dram_tensor("ksc", [128, 128], F32, kind="ExternalInput")
    trit = nc.dram_tensor("trit", [128, 128], BF16, kind="ExternalInput")
    ident_in = nc.dram_tensor("ident", [128, 128], BF16, kind="ExternalInput")
    if n_masks:
        maskt = nc.dram_tensor("maskt", [n_masks, 128, 512], BF16,
                               kind="ExternalInput")
    y = nc.dram_tensor("y", [S, D], BF16, kind="ExternalOutput")

    AF = mybir.ActivationFunctionType
    from contextlib import ExitStack
    with tile.TileContext(nc) as tc, ExitStack() as es:
        singles = es.enter_context(tc.tile_pool(name="singles", bufs=1))
        xpool = es.enter_context(tc.tile_pool(name="xpool", bufs=2))
        pwork = es.enter_context(tc.tile_pool(name="pwork", bufs=2))
        psum = es.enter_context(tc.tile_pool(name="psum", bufs=1, space="PSUM"))
        awork = es.enter_context(tc.tile_pool(name="awork", bufs=3, space="SBUF"))

        # ---- resident constants / weights; order = startup DMA priority ----
        wq_sb = singles.tile([128, NT, 512], BF16)
        wq_r = wq.ap().rearrange("(a p) n -> p a n", p=128)
        nc.sync.dma_start(out=wq_sb[:, 0:8, :], in_=wq_r[:, 0:8, :])

        xch = {}

        def load_xchunk(c):
            t = xpool.tile([128, NT, 256], BF16, tag="xch", name=f"xch_{c}")
            nc.sync.dma_start(
                out=t, in_=xT.ap()[:, c * 256:(c + 1) * 256]
                .rearrange("(a p) m -> p a m", p=128))
            xch[c] = t

        load_xchunk(0)
        nc.sync.dma_start(out=wq_sb[:, 8:16, :], in_=wq_r[:, 8:16, :])
        qsc_sb = singles.tile([128, 512], F32)
        nc.sync.dma_start(out=qsc_sb, in_=qsc.ap())
        ksc_sb = singles.tile([128, 128], F32)
        nc.sync.dma_start(out=ksc_sb, in_=ksc.ap())
        wkv_sb = singles.tile([128, NT, 256], BF16)
        nc.sync.dma_start(out=wkv_sb, in_=wkv.ap().rearrange("(a p) n -> p a n", p=128))
        load_xchunk(1)
        cos_sb = singles.tile([128, NT, HD], BF16)
        nc.sync.dma_start(out=cos_sb, in_=cosd.ap().rearrange("(a p) n -> p a n", p=128))
        sin_sb = singles.tile([128, NT, HD], BF16)
        nc.sync.dma_start(out=sin_sb, in_=sind.ap().rearrange("(a p) n -> p a n", p=128))
        ident_sb = singles.tile([128, 128], BF16)
        nc.sync.dma_start(out=ident_sb, in_=ident_in.ap())
        tri_sb = singles.tile([128, 128], BF16)
        nc.sync.dma_start(out=tri_sb, in_=trit.ap())
        wg_sb = singles.tile([128, NT, 512], BF16)
        nc.sync.dma_start(out=wg_sb, in_=wg.ap().rearrange("(a p) n -> p a n", p=128))
        if n_masks:
            mask_sb = singles.tile([128, n_masks, 512], BF16)
            nc.sync.dma_start(out=mask_sb,
                              in_=maskt.ap().rearrange("a p n -> p a n"))
        # wo load deferred (emitted before the first out-projection) so it
        # does not block the x-chunk streaming DMAs at startup
        wo_sb = singles.tile([128, 4, D], BF16)
        wo_loaded = []

        def load_wo():
            if not wo_loaded:
                nc.sync.dma_start(
                    out=wo_sb, in_=wo.ap().rearrange("(a p) n -> p a n", p=128))
                wo_loaded.append(True)

        qT = singles.tile([128, 4, S], BF16)       # head nt @0-63, 4+nt @64-127
        kT = singles.tile([128, S], BF16)          # group0 @0-63, group1 @64-127
        vext = singles.tile([128, 2, NT, 65], BF16)  # [v(64) | ones]
        nc.vector.memset(vext[:, :, :, 64], 1.0)
        eps_sb = singles.tile([128, 1], F32)
        nc.vector.memset(eps_sb, float(EPS))
        ones_sb = singles.tile([128, 64], BF16)
        nc.vector.memset(ones_sb, 1.0)
        graw = singles.tile([128, 4, S], BF16)     # gate^T; sigmoid in-place

        def emit_g_chains(i, xt):
            for nt in range(4):
                g_ps = psum.tile([128, 128], F32, tag="ps_to", bufs=2,
                                 name=f"gps_{i}_{nt}")
                for dt_ in range(NT):
                    mm(g_ps, wg_sb[:, dt_, nt * 128:(nt + 1) * 128],
                       rhs=xt[:, dt_, :],
                       start=(dt_ == 0), stop=(dt_ == NT - 1))
                nc.scalar.copy(graw[:, nt, i * 128:(i + 1) * 128], g_ps)

        def phase_p_tile(i, defer_g=False):
            """projections + rmsnorm + rope (rstd deferred) for s-tile i."""
            c, half = i // 2, i % 2
            if half == 0 and c + 1 < S // 256 and (c + 1) not in xch:
                load_xchunk(c + 1)
            xt = xch[c][:, :, half * 128:(half + 1) * 128]

            q_ps = psum.tile([128, 512], F32, tag="ps_qp", bufs=1, name=f"qps_{i}")
            for dt_ in range(NT):
                mm(q_ps, xt[:, dt_, :], rhs=wq_sb[:, dt_, :],
                   start=(dt_ == 0), stop=(dt_ == NT - 1))
            kv_ps = psum.tile([128, 256], F32, tag="ps_to", bufs=2, name=f"kvps_{i}")
            for dt_ in range(NT):
                mm(kv_ps, xt[:, dt_, :], rhs=wkv_sb[:, dt_, :],
                   start=(dt_ == 0), stop=(dt_ == NT - 1))
            if not defer_g:
                emit_g_chains(i, xt)

            # ---- q: square+reduce (rstd later), scale, rope ----
            q3 = q_ps.rearrange("p (h e) -> p h e", e=64)
            sq = pwork.tile([128, 8, 64], BF16, tag="sq")
            nc.scalar.square(sq, q3)
            nc.vector.reduce_sum(ssq[:, i % 4, :], sq, axis=mybir.AxisListType.X)
            qn = pwork.tile([128, 8, 64], BF16, tag="qn")
            nc.vector.tensor_mul(qn.rearrange("p h e -> p (h e)"), q_ps, qsc_sb)
            rot = pwork.tile([128, 8, 64], BF16, tag="rot")
            nc.vector.tensor_scalar_mul(rot[:, :, 0:32], qn[:, :, 32:64], -1.0)
            nc.vector.tensor_copy(rot[:, :, 32:64], qn[:, :, 0:32])
            qf = qfc[i % 4]
            for h in range(8):
                nc.vector.tensor_mul(qf[:, h, :], qn[:, h, :], cos_sb[:, i, :])
                nc.vector.tensor_mul(rot[:, h, :], rot[:, h, :], sin_sb[:, i, :])
            nc.vector.tensor_add(qf.rearrange("p h e -> p (h e)"),
                                 qf.rearrange("p h e -> p (h e)"),
                                 rot.rearrange("p h e -> p (h e)"))

            # ---- k ----
            k3 = kv_ps[:, 0:128].rearrange("p (h e) -> p h e", e=64)
            ksq = pwork.tile([128, 2, 64], BF16, tag="ksq")
            nc.scalar.square(ksq, k3)
            nc.vector.reduce_sum(ssk[:, i % 4, :], ksq, axis=mybir.AxisListType.X)
            kn = pwork.tile([128, 2, 64], BF16, tag="kn")
            nc.vector.tensor_mul(kn.rearrange("p h e -> p (h e)"),
                                 kv_ps[:, 0:128], ksc_sb)
            krot = pwork.tile([128, 2, 64], BF16, tag="krot")
            nc.vector.tensor_scalar_mul(krot[:, :, 0:32], kn[:, :, 32:64], -1.0)
            nc.vector.tensor_copy(krot[:, :, 32:64], kn[:, :, 0:32])
            kf = kfc[i % 4]
            for h in range(2):
                nc.vector.tensor_mul(kf[:, h, :], kn[:, h, :], cos_sb[:, i, :])
                nc.vector.tensor_mul(krot[:, h, :], krot[:, h, :], sin_sb[:, i, :])
            nc.vector.tensor_add(kf.rearrange("p h e -> p (h e)"),
                                 kf.rearrange("p h e -> p (h e)"),
                                 krot.rearrange("p h e -> p (h e)"))

            # v into v_ext (cast to bf16)
            nc.scalar.copy(vext[:, 0, i, 0:64], kv_ps[:, 128:192])
            nc.scalar.copy(vext[:, 1, i, 0:64], kv_ps[:, 192:256])
            return xt

        def phase_p_chunk_tail(qc):
            """sigmoid + batched rsqrt + rstd application + transposes."""
            # gate sigmoid first: depends only on graw, and the g1-low DMA
            # (engines cannot cross partitions) comes off the critical path
            gsl = graw[:, :, qc * 512:(qc + 1) * 512]
            nc.scalar.activation(gsl, gsl, AF.Sigmoid)
            gl = awork.tile([64, 4, 512], BF16, tag="glow", bufs=2,
                            name=f"glow_{qc}")
            nc.sync.dma_start(out=gl,
                              in_=graw[64:128, :, qc * 512:(qc + 1) * 512])
            nc.scalar.activation(ssq, ssq, AF.Sqrt, bias=eps_sb, scale=1.0 / 64)
            nc.scalar.activation(ssk, ssk, AF.Sqrt, bias=eps_sb, scale=1.0 / 64)
            nc.vector.reciprocal(ssq, ssq)
            nc.vector.reciprocal(ssk, ssk)
            for u in range(4):
                i = qc * 4 + u
                qf, kf = qfc[u], kfc[u]
                for h in range(8):
                    nc.vector.tensor_scalar_mul(qf[:, h, :], qf[:, h, :],
                                                ssq[:, u, h:h + 1])
                for h in range(2):
                    nc.vector.tensor_scalar_mul(kf[:, h, :], kf[:, h, :],
                                                ssk[:, u, h:h + 1])
                qf2 = qf.rearrange("p h e -> p (h e)")
                for nt in range(4):
                    tp = psum.tile([128, 128], BF16, tag="ps_s", bufs=3,
                                   name=f"tp_{i}_{nt}")
                    nc.tensor.transpose(tp, qf2[:, nt * 128:(nt + 1) * 128],
                                        ident_sb)
                    nc.scalar.copy(qT[:, nt, i * 128:(i + 1) * 128], tp)
                kf2 = kf.rearrange("p h e -> p (h e)")
                tpk = psum.tile([128, 128], BF16, tag="ps_s", bufs=3,
                                name=f"tpk_{i}")
                nc.tensor.transpose(tpk, kf2, ident_sb)
                nc.scalar.copy(kT[:, i * 128:(i + 1) * 128], tpk)
            return gl

        def a_head(qc, h, ctxg, gl):
            """attention for one head of q-chunk qc."""
            g, nt = h // 4, h % 4
            base = 64 * g
            kts = [kt for kt in range(NT) if classes[qc][kt] != "skip"]
            ctx_ps = psum.tile([128, 512], F32, tag="ps_ctx", bufs=2,
                               name=f"ctx_{qc}_{h}")

            def emit_score(j):
                kt = kts[j]
                cls = classes[qc][kt]
                lo = cls[1] if cls[0] == "tri" else 0
                s_ps = psum.tile([128, 512], F32, tag="ps_s", bufs=3,
                                 name=f"sps_{qc}_{h}_{kt}")
                mm(s_ps[:, lo:512],
                   kT[base:base + 64, kt * 128:(kt + 1) * 128],
                   rhs=qT[base:base + 64, nt, qc * 512 + lo:(qc + 1) * 512],
                   start=True, stop=True)
                eT = awork.tile([128, 512], BF16, tag="eT", bufs=4)
                nc.scalar.activation(eT[:, lo:512], s_ps[:, lo:512], AF.Exp)
                if cls[0] == "tri":
                    nc.vector.tensor_mul(eT[:, lo:lo + 128],
                                         eT[:, lo:lo + 128], tri_sb)
                elif cls[0] == "mask":
                    nc.vector.tensor_mul(eT, eT, mask_sb[:, cls[1], :])
                return eT, lo

            def emit_ctx(j, eT, lo):
                mm(ctx_ps[0:65, lo:512], vext[:, g, kts[j], :],
                   rhs=eT[:, lo:512], start=(j == 0), stop=(j == len(kts) - 1))

            # scores emitted one step ahead of the ctx accumulation so the
            # PE stream never head-of-line blocks on an exp in flight
            pend = None
            for j in range(len(kts)):
                cur = emit_score(j)
                if pend is not None:
                    emit_ctx(j - 1, *pend)
                pend = cur
            emit_ctx(len(kts) - 1, *pend)

            def normalize():
                # normalize + gate (partitions 0-63; denom at row 64)
                rstage = awork.tile([128, 512], BF16, tag="rstage", bufs=2)
                with nc.allow_low_precision(reason="bf16 softmax denom"):
                    nc.vector.reciprocal(rstage[64:65, :], ctx_ps[64:65, :])
                rb_ps = psum.tile([128, 512], F32, tag="ps_s", bufs=3,
                                  name=f"rbps_{qc}_{h}")
                mm(rb_ps[0:64, :], ones_sb[64:65, :],
                   rhs=rstage[64:65, :], start=True, stop=True)
                rbb = awork.tile([64, 512], BF16, tag="rbb", bufs=2)
                nc.vector.tensor_copy(rbb, rb_ps[0:64, :])
                ctxb = awork.tile([64, 512], BF16, tag="ctxb", bufs=2)
                nc.vector.tensor_copy(ctxb, ctx_ps[0:64, :])
                m1 = awork.tile([128, 512], BF16, tag="m1", bufs=2)
                gsl = (gl[:, nt, :] if g == 1
                       else graw[0:64, nt, qc * 512:(qc + 1) * 512])
                nc.vector.tensor_mul(m1[0:64, :], rbb, gsl)
                if g == 0:
                    nc.vector.tensor_mul(ctxg[nt][0:64, :], ctxb, m1[0:64, :])
                else:
                    tmp2 = awork.tile([64, 512], BF16, tag="tmp2", bufs=2)
                    nc.vector.tensor_mul(tmp2, ctxb, m1[0:64, :])
                    nc.sync.dma_start(out=ctxg[nt][64:128, :], in_=tmp2)
            return normalize

        def a_outproj_ssub(qc, ctxg, ssub):
            """output projection for one 128-row slab of q-chunk qc."""
            srow = qc * 512 + ssub * 128
            ostage = awork.tile([128, D], BF16, tag="ostage", bufs=3)
            for dc in range(4):
                o_ps = psum.tile([128, 512], F32, tag="ps_to", bufs=2,
                                 name=f"ops_{qc}_{ssub}_{dc}")
                for nt in range(4):
                    mm(o_ps, ctxg[nt][:, ssub * 128:(ssub + 1) * 128],
                       rhs=wo_sb[:, nt, dc * 512:(dc + 1) * 512],
                       start=(nt == 0), stop=(nt == 3))
                # last chunk runs after all exps: ACT is idle, share the
                # psum drains between both engines there
                if qc == NQC - 1 and dc % 2 == 0:
                    nc.scalar.copy(ostage[:, dc * 512:(dc + 1) * 512], o_ps)
                else:
                    nc.vector.tensor_copy(ostage[:, dc * 512:(dc + 1) * 512],
                                          o_ps)
            nc.sync.dma_start(out=y.ap()[srow:srow + 128, :], in_=ostage)

        # ======== main schedule: P chunk -> (tail) -> A chunk, interleaved ==
        ssq = singles.tile([128, 4, 8], F32)
        ssk = singles.tile([128, 4, 2], F32)
        qfc = [singles.tile([128, 8, 64], BF16, name=f"qfc{u}") for u in range(4)]
        kfc = [singles.tile([128, 2, 64], BF16, name=f"kfc{u}") for u in range(4)]

        # A(qc) heads interleaved with P tiles of chunk qc+1 and the
        # out-projection slabs of chunk qc-1, so the PE always has dense
        # projection matmuls to chew on while ACT runs the exps. g1 heads
        # first: their ctxg partition-shift DMAs overlap the later heads.
        # Chunk 0's gate chains are deferred past the q/kv chains so the PE
        # is not stalled on the wg weight DMA at startup.
        xts0 = [phase_p_tile(u, defer_g=True) for u in range(4)]
        for u in range(4):
            emit_g_chains(u, xts0[u])
        gl = phase_p_chunk_tail(0)
        HORDER = (4, 5, 6, 7, 0, 1, 2, 3)
        prev = None  # (qc, ctxg) awaiting out-projection
        for qc in range(NQC):
            load_wo()
            ctxg = [awork.tile([128, 512], BF16, tag=f"ctxg{nt}",
                               name=f"ctxg{nt}_{qc}", bufs=2)
                    for nt in range(4)]
            pend_norm = None
            for u, h in enumerate(HORDER):
                norm = a_head(qc, h, ctxg, gl)
                if pend_norm is not None:
                    pend_norm()
                pend_norm = norm
                if qc + 1 < NQC and u < 4:
                    phase_p_tile((qc + 1) * 4 + u)
                if prev is not None and u % 2 == 1:
                    a_outproj_ssub(prev[0], prev[1], u // 2)
            pend_norm()
            # tail of the NEXT chunk ahead of the trailing out-projections:
            # they are PE-dense with no ACT work, hiding the whole
            # sigmoid/sqrt/transpose chain (and its table switches)
            if qc + 1 < NQC:
                gl = phase_p_chunk_tail(qc + 1)
            prev = (qc, ctxg)
        for ssub in range(4):
            a_outproj_ssub(prev[0], prev[1], ssub)

    nc.compile()
    return nc


# ======================== host-side runner =================================
_CACHE = {}


class _Runner:
    """Jitted sharded executable for a prebuilt Bass module, reusable."""

    def __init__(self, nc, n_cores):
        import jax
        import numpy as _np
        from jax.sharding import Mesh, PartitionSpec
        from jax.experimental.shard_map import shard_map
        from concourse.bass2jax import (_bass_exec_p, partition_id_tensor,
                                        install_neuronx_cc_hook)
        install_neuronx_cc_hook()
        self.jax = jax
        self.nc = nc
        self.n_cores = n_cores
        partition_name = (nc.partition_id_tensor.name
                          if nc.partition_id_tensor else None)
        in_names, out_names, out_avals = [], [], []
        for alloc in nc.m.functions[0].allocations:
            if not isinstance(alloc, mybir.MemoryLocationSet):
                continue
            name = alloc.memorylocations[0].name
            if alloc.kind == "ExternalInput":
                if name != partition_name:
                    in_names.append(name)
            elif alloc.kind == "ExternalOutput":
                out_names.append(name)
                out_avals.append(jax.core.ShapedArray(
                    tuple(alloc.tensor_shape), mybir.dt.np(alloc.dtype)))
        self.in_names, self.out_names, self.out_avals = in_names, out_names, out_avals
        n_params = len(in_names)
        all_in = list(in_names) + list(out_names)
        if partition_name is not None:
            all_in.append(partition_name)
        self._dbg_name = nc.dbg_addr.name if nc.dbg_addr is not None else None

        def _body(*args):
            operands = list(args)
            if partition_name is not None:
                operands.append(partition_id_tensor())
            outs = _bass_exec_p.bind(
                *operands, out_avals=tuple(out_avals), in_names=tuple(all_in),
                out_names=tuple(out_names), lowering_input_output_aliases=(),
                sim_require_finite=True, sim_require_nnan=True, nc=nc)
            return tuple(outs)

        devices = jax.devices()[:n_cores]
        self.mesh = Mesh(_np.asarray(devices), ("core",))
        in_specs = (PartitionSpec("core"),) * (n_params + len(out_names))
        out_specs = (PartitionSpec("core"),) * len(out_names)
        self.fn = jax.jit(shard_map(_body, mesh=self.mesh, in_specs=in_specs,
                                    out_specs=out_specs, check_rep=False))
        self.dev_in = None

    def prepare(self, in_maps):
        import numpy as _np
        from jax.sharding import NamedSharding, PartitionSpec
        if self._dbg_name is not None:
            in_maps = [{**m, self._dbg_name: _np.zeros((1, 2), _np.uint32)}
                       for m in in_maps]
        concat = [_np.concatenate([_np.asarray(in_maps[c][n])
                                   for c in range(self.n_cores)], axis=0)
                  for n in self.in_names]
        # zero output buffers: device-resident, NOT donated, reused each run.
        # Valid because the kernel writes every element of its outputs.
        concat += [_np.zeros((self.n_cores * av.shape[0], *av.shape[1:]),
                             av.dtype) for av in self.out_avals]
        shard = NamedSharding(self.mesh, PartitionSpec("core"))
        self.dev_in = [self.jax.device_put(a, shard) for a in concat]
        return self

    def run(self):
        return self.jax.block_until_ready(self.fn(*self.dev_in))

    def results(self, outs):
        import numpy as _np
        res = []
        for c in range(self.n_cores):
            d = {}
            for i, name in enumerate(self.out_names):
                full = _np.asarray(outs[i])
                d[name] = full.reshape(self.n_cores, *self.out_avals[i].shape)[c]
            res.append(d)
        return res


def make_runner(nc, n_cores):
    return _Runner(nc, n_cores)


def _prep_core_inputs(inputs, b, t, shared):
    x = inputs["x"]
    import ml_dtypes
    bf = ml_dtypes.bfloat16

    if ("xT", b) not in shared:
        shared[("xT", b)] = np.ascontiguousarray(np.asarray(x[b]).T).astype(bf)
    if ("w", t) not in shared:
        Wq, Wk, Wv, Wg, Wo = (np.asarray(inputs[k])
                              for k in ("Wq", "Wk", "Wv", "Wg", "Wo"))
        heads = [8 * t + p for p in _PERM]
        qcols = np.concatenate([np.arange(h * 64, (h + 1) * 64) for h in heads])
        groups = [2 * t, 2 * t + 1]
        kcols = np.concatenate([np.arange(g * 64, (g + 1) * 64) for g in groups])
        shared[("w", t)] = {
            "wq": np.ascontiguousarray(Wq[:, qcols]).astype(bf),
            "wkv": np.ascontiguousarray(
                np.concatenate([Wk[:, kcols], Wv[:, kcols]], axis=1)).astype(bf),
            "wg": np.ascontiguousarray(Wg[:, qcols]).astype(bf),
            "wo": np.ascontiguousarray(Wo[qcols, :]).astype(bf),
        }
    if "const" not in shared:
        q_scale, k_scale = np.asarray(inputs["q_scale"]), np.asarray(inputs["k_scale"])
        cos, sin = np.asarray(inputs["cos"]), np.asarray(inputs["sin"])
        scaling = float(HD) ** -0.5
        tri = (np.arange(128)[:, None] <= np.arange(128)[None, :])
        shared["const"] = {
            "cosd": cos.astype(bf), "sind": sin.astype(bf),
            "qsc": np.broadcast_to(
                np.tile((1.0 + q_scale) * scaling, 8)[None, :],
                (128, 512)).astype(np.float32).copy(),
            "ksc": np.broadcast_to(
                np.tile(1.0 + k_scale, 2)[None, :],
                (128, 128)).astype(np.float32).copy(),
            "trit": tri.astype(bf),
            "ident": np.eye(128, dtype=np.float32).astype(bf),
        }
    return {"xT": shared[("xT", b)], **shared[("w", t)], **shared["const"]}


def kernel(**inputs):
    mask = np.asarray(inputs["mask"])
    classes, tiles = classify_mask(mask)
    key = mask.tobytes()
    if key not in _CACHE:
        nc = build_program(classes, len(tiles))
        _CACHE[key] = (nc, make_runner(nc, NCORES))
    nc, runner = _CACHE[key]

    import ml_dtypes
    mask_arr = (np.stack(tiles).astype(ml_dtypes.bfloat16) if tiles else None)
    shared = {}
    in_maps = []
    for c in range(NCORES):
        m = _prep_core_inputs(inputs, c // 4, c % 4, shared)
        if mask_arr is not None:
            m["maskt"] = mask_arr
        in_maps.append(m)

    runner.prepare(in_maps)
    outs = runner.run()
    res = runner.results(outs)
    out = np.zeros((B, S, D), np.float32)
    for c in range(NCORES):
        out[c // 4] += res[c]["y"].astype(np.float32)
    return out.astype(np.asarray(inputs["x"]).dtype)
